# revision 1
# baseline (speedup 1.0000x reference)
"""Trainium2 Bass kernel for nn_LinearUnit_65867618452250.

Single-step diagonal complex linear recurrence (LRU cell):
    out[b, j] = state[b, j] * a[j] + (inputs[b,0] + inputs[b,1]) * bcol[j]
with a = cat(as_real[:S], as_imag[:S]), bcol = cat(bs_real[:S], bs_imag[:S]).

Device strategy (data-parallel over batch, 8 NeuronCores):
  Rewrite   out = (state + s[b] * r[j]) * a[j]   with r = bcol / a
  (r precomputed on host in float64; where a == 0, r := 0).
  Per core shard (512, 8192) f32:
    - broadcast r and a across the 128 SBUF partitions once via a PE
      ones-outer-product (no HBM broadcast traffic),
    - per [128, 2048] chunk: HWDGE load -> one fused DVE
      scalar_tensor_tensor (s*r + state) -> one DVE tensor_tensor
      multiply by a -> HWDGE store.
  Memory-bound: 16 MB in + 16 MB out per core.
"""

import numpy as np

import concourse.bacc as bacc
import concourse.mybir as mybir
from concourse import tile
from concourse.bass_utils import run_bass_kernel_spmd

N_CORES = 8
BATCH = 4096
NU = 8192                # num_units = 2S
P = 128                  # SBUF partitions
B_CORE = BATCH // N_CORES   # 512 rows per core
T_TILES = B_CORE // P       # 4 batch tiles per core
FCHUNK = 2048               # free-dim chunk (1 MB tiles)
N_CH = NU // FCHUNK         # 4
BC = 512                    # broadcast matmul width (one PSUM bank)
F32 = mybir.dt.float32

# Set by test harness to capture an NTFF profile; kernel() records the
# measured exec time in LAST.
TRACE = False
LAST = {}

_nc = None


def _build():
    global _nc
    if _nc is not None:
        return _nc
    nc = bacc.Bacc("TRN2", target_bir_lowering=False, debug=False,
                   num_devices=N_CORES)
    state = nc.dram_tensor("state", [B_CORE, NU], F32, kind="ExternalInput")
    s_col = nc.dram_tensor("s_col", [P, T_TILES], F32, kind="ExternalInput")
    r_row = nc.dram_tensor("r_row", [1, NU], F32, kind="ExternalInput")
    a_row = nc.dram_tensor("a_row", [1, NU], F32, kind="ExternalInput")
    out = nc.dram_tensor("out", [B_CORE, NU], F32, kind="ExternalOutput")
    AOT = mybir.AluOpType

    with tile.TileContext(nc) as tc:
        with (
            tc.tile_pool(name="consts", bufs=1) as cpool,
            tc.tile_pool(name="vrows", bufs=4) as vpool,
            tc.tile_pool(name="psum", bufs=4, space="PSUM") as ppool,
            tc.tile_pool(name="work", bufs=4) as wpool,
        ):
            s_sb = cpool.tile([P, T_TILES], F32)
            nc.sync.dma_start(s_sb[:], s_col[:])
            ones = cpool.tile([1, P], F32)
            nc.any.memset(ones[:], 1.0)

            # Broadcast r and a across all 128 partitions: psum = ones^T @ vec
            R_b = cpool.tile([P, NU], F32)
            A_b = cpool.tile([P, NU], F32)
            for dram_vec, dst in ((r_row, R_b), (a_row, A_b)):
                for c in range(N_CH):
                    rv = vpool.tile([1, FCHUNK], F32, tag="vrow")
                    nc.sync.dma_start(
                        rv[:], dram_vec[0:1, c * FCHUNK:(c + 1) * FCHUNK])
                    for j in range(FCHUNK // BC):
                        ps = ppool.tile([P, BC], F32, tag="bc")
                        nc.tensor.matmul(ps[:], ones[:],
                                         rv[0:1, j * BC:(j + 1) * BC])
                        col = c * FCHUNK + j * BC
                        nc.scalar.copy(dst[:, col:col + BC], ps[:])

            for t in range(T_TILES):
                rows = slice(t * P, (t + 1) * P)
                for c in range(N_CH):
                    cs = slice(c * FCHUNK, (c + 1) * FCHUNK)
                    st = wpool.tile([P, FCHUNK], F32, tag="st")
                    nc.sync.dma_start(st[:], state[rows, cs])
                    tmp = wpool.tile([P, FCHUNK], F32, tag="tmp")
                    nc.vector.scalar_tensor_tensor(
                        tmp[:], R_b[:, cs], s_sb[:, t:t + 1], st[:],
                        op0=AOT.mult, op1=AOT.add)
                    o = wpool.tile([P, FCHUNK], F32, tag="o")
                    nc.vector.tensor_tensor(o[:], tmp[:], A_b[:, cs],
                                            op=AOT.mult)
                    nc.scalar.dma_start(out[rows, cs], o[:])

    nc.compile()
    _nc = nc
    return nc


def kernel(inputs, state, as_real, as_imag, bs_real, bs_imag):
    inputs = np.asarray(inputs, dtype=np.float32)
    state = np.ascontiguousarray(np.asarray(state, dtype=np.float32))
    as_real = np.asarray(as_real, dtype=np.float32)
    as_imag = np.asarray(as_imag, dtype=np.float32)
    bs_real = np.asarray(bs_real, dtype=np.float32)
    bs_imag = np.asarray(bs_imag, dtype=np.float32)

    S = as_real.shape[0] // 2
    a = np.concatenate([as_real[:S], as_imag[:S]]).astype(np.float64)
    b = np.concatenate([bs_real[:S], bs_imag[:S]]).astype(np.float64)
    safe_a = np.where(a == 0.0, 1.0, a)
    r = np.where(a == 0.0, 0.0, b / safe_a).astype(np.float32)
    a32 = a.astype(np.float32)
    s = (inputs[:, 0] + inputs[:, 1]).astype(np.float32)   # (BATCH,)

    nc = _build()

    r_rowv = np.ascontiguousarray(r.reshape(1, NU))
    a_rowv = np.ascontiguousarray(a32.reshape(1, NU))
    in_maps = []
    for c in range(N_CORES):
        sh = np.ascontiguousarray(state[c * B_CORE:(c + 1) * B_CORE])
        sc = np.ascontiguousarray(
            s[c * B_CORE:(c + 1) * B_CORE].reshape(T_TILES, P).T)
        in_maps.append({"state": sh, "s_col": sc,
                        "r_row": r_rowv, "a_row": a_rowv})

    res = run_bass_kernel_spmd(nc, in_maps, list(range(N_CORES)),
                               trace=TRACE)
    LAST["exec_time_ns"] = res.exec_time_ns
    LAST["res"] = res

    full = np.concatenate(
        [res.results[i]["out"] for i in range(N_CORES)], axis=0)
    return full, full


# revision 2
# speedup vs baseline: 1.5367x; 1.5367x over previous
"""Trainium2 Bass kernel for nn_LinearUnit_65867618452250.

Single-step diagonal complex linear recurrence (LRU cell):
    out[b, j] = state[b, j] * a[j] + (inputs[b,0] + inputs[b,1]) * bcol[j]
with a = cat(as_real[:S], as_imag[:S]), bcol = cat(bs_real[:S], bs_imag[:S]).

Device strategy (data-parallel over batch, 8 NeuronCores):
  Rewrite   out = (state + s[b] * r[j]) * a[j]   with r = bcol / a
  (r precomputed on host in float64; where a == 0, r := 0).
  Per core shard (512, 8192) f32:
    - broadcast r and a across the 128 SBUF partitions via PE: each f32
      vector is split host-side into 3 bf16 components summing exactly to
      the f32 value; one K=3 matmul against a bf16 ones matrix reconstructs
      the exact f32 broadcast in PSUM (fast: bf16 streaming, single
      LDWEIGHTS; no HBM broadcast traffic),
    - per [128, 2048] chunk: HWDGE load -> one fused DVE
      scalar_tensor_tensor (s*r + state) -> one DVE tensor_tensor
      multiply by a -> HWDGE store.
  Loop is column-chunk-outer so compute starts as soon as the first
  broadcast chunk lands. Memory-bound: 16 MB in + 16 MB out per core.
"""

import numpy as np
import ml_dtypes

import concourse.bacc as bacc
import concourse.mybir as mybir
from concourse import tile
from concourse.bass_utils import run_bass_kernel_spmd

N_CORES = 8
BATCH = 4096
NU = 8192                # num_units = 2S
P = 128                  # SBUF partitions
B_CORE = BATCH // N_CORES   # 512 rows per core
T_TILES = B_CORE // P       # 4 batch tiles per core
FCHUNK = 2048               # free-dim chunk (1 MB tiles)
N_CH = NU // FCHUNK         # 4
BC = 512                    # broadcast matmul width (one PSUM bank)
F32 = mybir.dt.float32
BF16 = mybir.dt.bfloat16

# Set by test harness to capture an NTFF profile; kernel() records the
# measured exec time in LAST.
TRACE = False
LAST = {}

_nc = None


def _build():
    global _nc
    if _nc is not None:
        return _nc
    nc = bacc.Bacc("TRN2", target_bir_lowering=False, debug=False,
                   num_devices=N_CORES)
    state = nc.dram_tensor("state", [B_CORE, NU], F32, kind="ExternalInput")
    s_col = nc.dram_tensor("s_col", [P, T_TILES], F32, kind="ExternalInput")
    r3_rows = nc.dram_tensor("r3_rows", [3, NU], BF16, kind="ExternalInput")
    a3_rows = nc.dram_tensor("a3_rows", [3, NU], BF16, kind="ExternalInput")
    out = nc.dram_tensor("out", [B_CORE, NU], F32, kind="ExternalOutput")
    AOT = mybir.AluOpType

    with tile.TileContext(nc) as tc:
        with (
            tc.tile_pool(name="consts", bufs=1) as cpool,
            tc.tile_pool(name="vrows", bufs=3) as vpool,
            tc.tile_pool(name="psum", bufs=4, space="PSUM") as ppool,
            tc.tile_pool(name="work", bufs=4) as wpool,
        ):
            s_sb = cpool.tile([P, T_TILES], F32)
            nc.sync.dma_start(s_sb[:], s_col[:])
            ones3 = cpool.tile([3, P], BF16)
            nc.any.memset(ones3[:], 1.0)

            R_b = cpool.tile([P, NU], F32)
            A_b = cpool.tile([P, NU], F32)

            for c in range(N_CH):
                cs = slice(c * FCHUNK, (c + 1) * FCHUNK)
                # Broadcast this column chunk of r and a across partitions:
                # psum = ones3.T @ [hi; mid; lo] == exact f32 vector values.
                for dram_vec, dst in ((r3_rows, R_b), (a3_rows, A_b)):
                    rv = vpool.tile([3, FCHUNK], BF16, tag="vrow")
                    nc.sync.dma_start(rv[:], dram_vec[0:3, cs])
                    for j in range(FCHUNK // BC):
                        ps = ppool.tile([P, BC], F32, tag="bc")
                        nc.tensor.matmul(ps[:], ones3[:],
                                         rv[0:3, j * BC:(j + 1) * BC])
                        col = c * FCHUNK + j * BC
                        nc.scalar.copy(dst[:, col:col + BC], ps[:])

                for t in range(T_TILES):
                    rows = slice(t * P, (t + 1) * P)
                    st = wpool.tile([P, FCHUNK], F32, tag="st", bufs=6)
                    nc.sync.dma_start(st[:], state[rows, cs])
                    tmp = wpool.tile([P, FCHUNK], F32, tag="tmp")
                    nc.vector.scalar_tensor_tensor(
                        tmp[:], R_b[:, cs], s_sb[:, t:t + 1], st[:],
                        op0=AOT.mult, op1=AOT.add)
                    o = wpool.tile([P, FCHUNK], F32, tag="o")
                    nc.vector.tensor_tensor(o[:], tmp[:], A_b[:, cs],
                                            op=AOT.mult)
                    nc.scalar.dma_start(out[rows, cs], o[:])

    nc.compile()
    _nc = nc
    return nc


def _split3_bf16(x32):
    """Split f32 vector into 3 bf16 rows summing exactly to x32."""
    bf = ml_dtypes.bfloat16
    hi = x32.astype(bf)
    rem = x32 - hi.astype(np.float32)
    mid = rem.astype(bf)
    rem2 = rem - mid.astype(np.float32)
    lo = rem2.astype(bf)
    return np.ascontiguousarray(np.stack([hi, mid, lo]))


def kernel(inputs, state, as_real, as_imag, bs_real, bs_imag):
    inputs = np.asarray(inputs, dtype=np.float32)
    state = np.ascontiguousarray(np.asarray(state, dtype=np.float32))
    as_real = np.asarray(as_real, dtype=np.float32)
    as_imag = np.asarray(as_imag, dtype=np.float32)
    bs_real = np.asarray(bs_real, dtype=np.float32)
    bs_imag = np.asarray(bs_imag, dtype=np.float32)

    S = as_real.shape[0] // 2
    a = np.concatenate([as_real[:S], as_imag[:S]]).astype(np.float64)
    b = np.concatenate([bs_real[:S], bs_imag[:S]]).astype(np.float64)
    safe_a = np.where(a == 0.0, 1.0, a)
    r = np.where(a == 0.0, 0.0, b / safe_a).astype(np.float32)
    a32 = a.astype(np.float32)
    s = (inputs[:, 0] + inputs[:, 1]).astype(np.float32)   # (BATCH,)

    nc = _build()

    r3 = _split3_bf16(r)
    a3 = _split3_bf16(a32)
    in_maps = []
    for c in range(N_CORES):
        sh = np.ascontiguousarray(state[c * B_CORE:(c + 1) * B_CORE])
        sc = np.ascontiguousarray(
            s[c * B_CORE:(c + 1) * B_CORE].reshape(T_TILES, P).T)
        in_maps.append({"state": sh, "s_col": sc,
                        "r3_rows": r3, "a3_rows": a3})

    res = run_bass_kernel_spmd(nc, in_maps, list(range(N_CORES)),
                               trace=TRACE)
    LAST["exec_time_ns"] = res.exec_time_ns
    LAST["res"] = res

    full = np.concatenate(
        [res.results[i]["out"] for i in range(N_CORES)], axis=0)
    return full, full
